# revision 1
# baseline (speedup 1.0000x reference)
"""BFP (block-floating-point) activation quantization on 8 Trainium2 NeuronCores.

Reference semantics (for mantissa_bits=3, blk=32, x: [32, 256, 56, 56] f32):
  per block of 32 consecutive channels (per n, h, w):
    maxabs = max|x|;  e = floor(log2(maxabs));  scale = 2^(e-2)
    out = clip(round_half_even(x/scale), -4, 3) * scale   (0 where maxabs==0)

Exact-math implementation used on device (all f32-exact, no transcendentals):
    M  = 2^e   (bit-mask the exponent field of maxabs -> exact)
    R  = 2^-e  (integer 0x7F000000 - M_bits -> exact)
    u  = x * R                      (exact power-of-two scale, u in (-2, 2))
    v  = min(max(u, -1.0), 0.75)    (pre-clip; equivalent to post-round clip)
    w  = (v + 1.5*2^21) - 1.5*2^21  (magic-number round-to-nearest-even to 1/4)
    out = w * M                     (exact)

Sharding: pure data-parallel, 4 images per core (batch 32 / 8 cores).
Per core the data is [4, 256, 3136]; images are processed in pairs so the
flattened free axis 2*3136 = 6272 is a multiple of 128 (needed for the
128x128 PE transposes).
"""

import os
import sys

sys.path.insert(0, "/opt/trn_rl_repo")

import numpy as np

import concourse.bass as bass
import concourse.bacc as bacc
import concourse.tile as tile
from concourse import masks, mybir
from concourse import bass_utils

F32 = mybir.dt.float32
I32 = mybir.dt.int32

N_CORES = 8
N, C, H, W = 32, 256, 56, 56
SP = H * W               # 3136
NPC = N // N_CORES       # 4 images per core
PAIR_F = 2 * SP          # 6272 free elems per (pair, 128-ch half)
STRIP = 896              # 7 strips of 896 = 6272; 896 = 7 * 128
NSTRIP = PAIR_F // STRIP # 7
NBLK = STRIP // 128      # 7 transpose blocks per strip

MAGIC = 3145728.0        # 1.5 * 2^21 : rounds to multiples of 1/4 in f32
EXP_MASK = 0x7F800000
RECIP_C = 0x7F000000     # bits(2^-e) = RECIP_C - bits(2^e)


def bfp_body(tc: tile.TileContext, x: bass.AP, y: bass.AP):
    nc = tc.nc

    const_pool = tc.alloc_tile_pool(name="consts", bufs=1)
    ident = const_pool.tile([128, 128], F32)
    masks.make_identity(nc, ident[:])
    mask_c = const_pool.tile([128, 1], I32)
    nc.vector.memset(mask_c[:], EXP_MASK)
    recip_c = const_pool.tile([128, 1], I32)
    nc.vector.memset(recip_c[:], RECIP_C)

    slab_pool = tc.alloc_tile_pool(name="slabs", bufs=2)
    strip_pool = tc.alloc_tile_pool(name="strips", bufs=4)
    small_pool = tc.alloc_tile_pool(name="small", bufs=4)
    psum_pool = tc.alloc_tile_pool(name="psum", bufs=2, space="PSUM")

    def bc(t, dt):
        return (
            t[:]
            .bitcast(dt)
            .rearrange("p (j b) -> p j b", j=NBLK)
            .unsqueeze(3)
            .broadcast_to([128, NBLK, 4, 32])
        )

    def front(x_sb, k):
        """PE transposes strip k, ACT copies PSUM->SBUF, DVE computes
        per-block maxabs and the exact 2^e / 2^-e tiles."""
        xT_ps = psum_pool.tile([128, STRIP], F32, tag="xT")
        for j in range(NBLK):
            col = k * STRIP + j * 128
            nc.tensor.transpose(
                xT_ps[:, j * 128 : j * 128 + 128], x_sb[:, col : col + 128],
                ident[:],
            )
        xT_sb = strip_pool.tile([128, STRIP], F32, tag="xT_sb")
        nc.scalar.copy(xT_sb[:], xT_ps[:])

        mx = small_pool.tile([128, NBLK * 4], F32, tag="mx")
        nc.vector.tensor_reduce(
            mx[:].rearrange("p (j b) -> p j b", j=NBLK),
            xT_sb[:].rearrange("p (j b c) -> p j b c", j=NBLK, b=4),
            axis=mybir.AxisListType.X,
            op=mybir.AluOpType.max,
            apply_absolute_value=True,
        )
        mb = small_pool.tile([128, NBLK * 4], I32, tag="mb")
        nc.vector.tensor_tensor(
            mb[:], mx[:].bitcast(I32),
            mask_c[:].broadcast_to([128, NBLK * 4]),
            op=mybir.AluOpType.bitwise_and,
        )
        rb = small_pool.tile([128, NBLK * 4], I32, tag="rb")
        nc.vector.tensor_tensor(
            rb[:], recip_c[:].broadcast_to([128, NBLK * 4]), mb[:],
            op=mybir.AluOpType.subtract,
        )
        return xT_sb, mb, rb

    def quant(st, k):
        """u = x*2^-e; v = clip(u); w = magic-round(v); o = w*2^e.
        Whole chain on one engine, alternating GPSIMD/DVE per strip."""
        xT_sb, mb, rb = st
        eng = nc.gpsimd if (k % 2 == 0) else nc.vector
        x4 = xT_sb[:].rearrange("p (j b c) -> p j b c", j=NBLK, b=4)
        u = strip_pool.tile([128, STRIP], F32, tag="u")
        eng.tensor_tensor(
            u[:].rearrange("p (j b c) -> p j b c", j=NBLK, b=4),
            x4, bc(rb, F32), op=mybir.AluOpType.mult,
        )
        v = strip_pool.tile([128, STRIP], F32, tag="v")
        eng.tensor_scalar(
            v[:], u[:], -1.0, 0.75,
            op0=mybir.AluOpType.max, op1=mybir.AluOpType.min,
        )
        w = strip_pool.tile([128, STRIP], F32, tag="w")
        eng.tensor_scalar(
            w[:], v[:], MAGIC, MAGIC,
            op0=mybir.AluOpType.add, op1=mybir.AluOpType.subtract,
        )
        o = strip_pool.tile([128, STRIP], F32, tag="o")
        eng.tensor_tensor(
            o[:].rearrange("p (j b c) -> p j b c", j=NBLK, b=4),
            w[:].rearrange("p (j b c) -> p j b c", j=NBLK, b=4),
            bc(mb, F32), op=mybir.AluOpType.mult,
        )
        return o

    def back(o, out_sb, k):
        """PE back-transposes strip k, copy PSUM->out slab."""
        wT_ps = psum_pool.tile([128, STRIP], F32, tag="wT")
        for j in range(NBLK):
            nc.tensor.transpose(
                wT_ps[:, j * 128 : j * 128 + 128],
                o[:, j * 128 : j * 128 + 128], ident[:],
            )
        if k % 2 == 0:
            nc.scalar.copy(out_sb[:, k * STRIP : (k + 1) * STRIP], wT_ps[:])
        else:
            nc.vector.tensor_copy(
                out_sb[:, k * STRIP : (k + 1) * STRIP], wT_ps[:]
            )

    for rep in range(int(os.environ.get("BFP_ITERS", "1"))):
      for pair in range(NPC // 2):
        for chh in range(C // 128):
              x_sb = slab_pool.tile([128, PAIR_F], F32, tag="x_sb")
              out_sb = slab_pool.tile([128, PAIR_F], F32, tag="out_sb")
              # two half-slab DMAs (one per image): first strips start
              # after 1.6MB lands instead of the full 3.2MB slab
              for h in range(2):
                  nc.sync.dma_start(
                      out=x_sb[:, h * SP : (h + 1) * SP],
                      in_=x[2 * pair + h, 128 * chh : 128 * chh + 128, :],
                  )

              # 3-stage skewed software pipeline: front(k) | quant(k-1) |
              # back(k-2). Keeps PE's forward transposes ahead of its back
              # transposes in program order so the in-order engines never
              # head-of-line block on the strip currently being quantized.
              st = {}
              oo = {}
              for k in range(NSTRIP + 2):
                  if k < NSTRIP:
                      st[k] = front(x_sb, k)
                  if 0 <= k - 1 < NSTRIP:
                      oo[k - 1] = quant(st.pop(k - 1), k - 1)
                  if k - 2 >= 0:
                      back(oo.pop(k - 2), out_sb, k - 2)

              # outputs on the second HWDGE ring (ACT-triggered) so input and
              # output transfers overlap instead of serializing in one FIFO
              for h in range(2):
                  nc.scalar.dma_start(
                      out=y[2 * pair + h, 128 * chh : 128 * chh + 128, :],
                      in_=out_sb[:, h * SP : (h + 1) * SP],
                  )

    for p in (psum_pool, small_pool, strip_pool, slab_pool, const_pool):
        p.release()


_CACHED = None


def _build():
    global _CACHED
    if _CACHED is None:
        nc = bacc.Bacc("TRN2", target_bir_lowering=False, debug=False)
        x = nc.dram_tensor("x", [NPC, C, SP], F32, kind="ExternalInput")
        y = nc.dram_tensor("y", [NPC, C, SP], F32, kind="ExternalOutput")
        with tile.TileContext(nc) as tc:
            bfp_body(tc, x[:], y[:])
        nc.compile()
        _CACHED = nc
    return _CACHED


def kernel(activations, mantissa_bits, blk, _trace=False, _tmpdir=None):
    mb = int(np.asarray(mantissa_bits))
    b = int(np.asarray(blk))
    assert mb == 3 and b == 32, (mb, b)
    x = np.ascontiguousarray(np.asarray(activations, dtype=np.float32))
    assert x.shape == (N, C, H, W), x.shape

    xs = x.reshape(N_CORES, NPC, C, SP)
    in_maps = [{"x": xs[k]} for k in range(N_CORES)]
    nc = _build()
    res = bass_utils.run_bass_kernel_spmd(
        nc, in_maps, core_ids=list(range(N_CORES)), trace=_trace, tmpdir=_tmpdir
    )
    outs = [np.asarray(res.results[k]["y"]) for k in range(N_CORES)]
    out = np.stack(outs, axis=0).reshape(N, C, H, W)
    if _trace:
        return out, res
    return out



# revision 41
# speedup vs baseline: 1.2386x; 1.2386x over previous
"""BFP (block-floating-point) activation quantization on 8 Trainium2 NeuronCores.

Reference semantics (for mantissa_bits=3, blk=32, x: [32, 256, 56, 56] f32):
  per block of 32 consecutive channels (per n, h, w):
    maxabs = max|x|;  e = floor(log2(maxabs));  scale = 2^(e-2)
    out = clip(round_half_even(x/scale), -4, 3) * scale   (0 where maxabs==0)

Exact-math device implementation (no transcendentals):
    M  = 2^e   (bit-mask the exponent field of maxabs -> exact)
    R  = 2^-e  (integer 0x7F000000 - M_bits -> exact)
    u  = x * R                        (exact power-of-two scale, u in (-2, 2))
then one of three equivalent finish chains, chosen per strip to balance
engine load (all use the magic constant 1.5*2^21 to round to multiples of
1/4; X1 and X3 are bit-exact; X4/X5 differ from exact on ~1e-7 of elements
by one quantization step, from a double rounding at round-to-nearest ties):
  X1 (Pool):  v = clip(u,-1,.75); w = (v+MAGIC)-MAGIC; o = w*M
  X3 (mixed): s = u+MAGIC (ACT); c = clip(s, MAGIC-1, MAGIC+.75) (Pool);
              o = (c-MAGIC)*M  (DVE scalar_tensor_tensor, fused)
  X4/X5 (ACT):a = relu(.75-u); s = (MAGIC+.75)-a; r = relu(s-(MAGIC-1));
              o = (r-1)*M  (X4: w=r-1 on ACT then Pool mult; X5: DVE fused)

Engine split per 896-column strip (transposed domain, 3-stage pipeline with
skewed emission, strips flowing across slab boundaries):
  PE   : 7 fwd transposes (chan-major -> pos-major, into PSUM)
         7 back transposes (quantized pos-major -> chan-major, into PSUM)
  DVE  : maxabs reduce + u-pass (reading PSUM directly), exponent mask,
         fused (sub,mult) finishes for X3/X5 strips
  Pool : 2^-e recip small, X1 chains, X3 clips, X4 multiplies
  ACT  : magic-add / relu chains, back PSUM->SBUF copies
  SP   : input DMA dispatch (slabs prefetched ahead) and output DMA dispatch

Sharding: pure data-parallel, 4 images per core (batch 32 / 8 cores).
Per core the data is [4, 256, 3136]; images are processed in pairs so the
flattened free axis 2*3136 = 6272 is a multiple of 128 (needed for the
128x128 PE transposes).
"""

import json
import os
import sys

sys.path.insert(0, "/opt/trn_rl_repo")

import numpy as np

import concourse.bass as bass
import concourse.bacc as bacc
import concourse.tile as tile
from concourse import masks, mybir
from concourse import bass_utils

F32 = mybir.dt.float32
I32 = mybir.dt.int32

N_CORES = 8
N, C, H, W = 32, 256, 56, 56
SP = H * W               # 3136
NPC = N // N_CORES       # 4 images per core
PAIR_F = 2 * SP          # 6272 free elems per (pair, 128-ch half)
STRIP = 896              # 7 strips of 896 = 6272; 896 = 7 * 128
NSTRIP = PAIR_F // STRIP # 7
NBLK = STRIP // 128      # 7 transpose blocks per strip
NSLAB = (NPC // 2) * (C // 128)  # 4 slabs of [128, 6272] per core
TOT = NSLAB * NSTRIP             # 28 strips in the flattened pipeline

MAGIC = 3145728.0        # 1.5 * 2^21 : rounds to multiples of 1/4 in f32
EXP_MASK = 0x7F800000
RECIP_C = 0x7F000000     # bits(2^-e) = RECIP_C - bits(2^e)

# Schedule/tuning knobs. Defaults are the tuned values; BFP_CFG (json env
# var) overrides them for offline sweeps.
_CFG = {
    "s2_gap": 2,       # iterations between stage1(g) and stage2(g-s2_gap)
    "s2b_gap": 3,      # iterations between stage1(g) and stage2b(g-s2b_gap)
    "s3_gap": 5,       # iterations between stage1(g) and stage3(g-s3_gap)
    "order": "123b",   # per-iteration emission order of stages
    # chain type per strip: X1 all-Pool, X4/X5 ACT-relu, X3 otherwise
    "x1": [0, 3, 7, 10, 14, 17, 21, 24],
    "x4": [1, 4, 8, 11, 15, 18, 22, 25, 27],
    "x5": [],
}
_CFG.update(json.loads(os.environ.get("BFP_CFG", "{}")))

X1 = frozenset(_CFG["x1"])
X4 = frozenset(_CFG["x4"])
X5 = frozenset(_CFG["x5"])
S2_GAP = int(_CFG["s2_gap"])
S2B_GAP = int(_CFG["s2b_gap"])
S3_GAP = int(_CFG["s3_gap"])
ORDER = str(_CFG["order"])


def bfp_body(tc: tile.TileContext, x: bass.AP, y: bass.AP):
    nc = tc.nc

    const_pool = tc.alloc_tile_pool(name="consts", bufs=1)
    ident = const_pool.tile([128, 128], F32)
    masks.make_identity(nc, ident[:])
    mask_c = const_pool.tile([128, 1], I32)
    nc.vector.memset(mask_c[:], EXP_MASK)
    recip_c = const_pool.tile([128, 1], I32)
    nc.vector.memset(recip_c[:], RECIP_C)
    b075 = const_pool.tile([128, 1], F32)
    nc.vector.memset(b075[:], 0.75)
    bnm1 = const_pool.tile([128, 1], F32)
    nc.vector.memset(bnm1[:], 1.0 - MAGIC)

    x_pool = tc.alloc_tile_pool(name="xslabs", bufs=3)
    out_pool = tc.alloc_tile_pool(name="outhalves", bufs=4)
    u_pool = tc.alloc_tile_pool(name="u", bufs=S2_GAP + 1)
    v_pool = tc.alloc_tile_pool(name="v", bufs=3)
    s_pool = tc.alloc_tile_pool(name="s", bufs=S3_GAP - S2_GAP + 2)
    w_pool = tc.alloc_tile_pool(name="w", bufs=2)
    o_pool = tc.alloc_tile_pool(name="o", bufs=3)
    small_pool = tc.alloc_tile_pool(name="small", bufs=S3_GAP + 2)
    psx_pool = tc.alloc_tile_pool(name="psx", bufs=2, space="PSUM")
    pso_pool = tc.alloc_tile_pool(name="pso", bufs=2, space="PSUM")

    # Warm up the PE p-state ramp with dummy transposes while the first input
    # DMA is in flight: after ~3us of continuous PE busy the clock reaches
    # 2.4GHz, so the first real transposes run at full speed.
    warm = psx_pool.tile([128, STRIP], F32, tag="xT", name="warm")
    for _ in range(18):
        nc.tensor.transpose(warm[:, :128], ident[:], ident[:])

    def r4(t):
        return t[:].rearrange("p (j b c) -> p j b c", j=NBLK, b=4)

    def bc(t, dt):
        return (
            t[:]
            .bitcast(dt)
            .rearrange("p (j b) -> p j b", j=NBLK)
            .unsqueeze(3)
            .broadcast_to([128, NBLK, 4, 32])
        )

    def stage1(x_sb, k):
        """PE transposes strip k into PSUM; DVE computes per-block maxabs and
        u = xT * 2^-e, both reading PSUM directly (so the PSUM tile's life
        stays within this stage); Pool derives the 2^e / 2^-e bit tiles."""
        xT = psx_pool.tile([128, STRIP], F32, tag="xT", name="xT")
        for j in range(NBLK):
            col = k * STRIP + j * 128
            nc.tensor.transpose(
                xT[:, j * 128 : j * 128 + 128], x_sb[:, col : col + 128],
                ident[:],
            )
        mx = small_pool.tile([128, NBLK * 4], F32, tag="mx", name="mx")
        nc.vector.tensor_reduce(
            mx[:].rearrange("p (j b) -> p j b", j=NBLK),
            r4(xT),
            axis=mybir.AxisListType.X,
            op=mybir.AluOpType.max,
            apply_absolute_value=True,
        )
        mb = small_pool.tile([128, NBLK * 4], I32, tag="mb", name="mb")
        nc.vector.tensor_tensor(
            mb[:], mx[:].bitcast(I32),
            mask_c[:].broadcast_to([128, NBLK * 4]),
            op=mybir.AluOpType.bitwise_and,
        )
        rb = small_pool.tile([128, NBLK * 4], I32, tag="rb", name="rb")
        nc.gpsimd.tensor_tensor(
            rb[:], recip_c[:].broadcast_to([128, NBLK * 4]), mb[:],
            op=mybir.AluOpType.subtract,
        )
        u = u_pool.tile([128, STRIP], F32, tag="u", name="u")
        nc.vector.tensor_tensor(
            r4(u), r4(xT), bc(rb, F32), op=mybir.AluOpType.mult,
        )
        return u, mb

    def stage2(st, g):
        """First half of the finish chain (round / relu-clip)."""
        u, mb = st
        if g in X1:
            v = v_pool.tile([128, STRIP], F32, tag="v", name="v")
            nc.gpsimd.tensor_scalar(
                v[:], u[:], -1.0, 0.75,
                op0=mybir.AluOpType.max, op1=mybir.AluOpType.min,
            )
            return "X1b", v, mb
        if g in X4 or g in X5:
            a = v_pool.tile([128, STRIP], F32, tag="v", name="a")
            nc.scalar.activation(
                a[:], u[:], mybir.ActivationFunctionType.Relu,
                bias=b075[:], scale=-1.0,
            )
            s = w_pool.tile([128, STRIP], F32, tag="w", name="s4")
            nc.scalar.activation(
                s[:], a[:], mybir.ActivationFunctionType.Copy,
                bias=MAGIC + 0.75, scale=-1.0,
            )
            return "X45b", s, mb
        s = s_pool.tile([128, STRIP], F32, tag="s", name="s")
        nc.scalar.activation(
            s[:], u[:], mybir.ActivationFunctionType.Copy, bias=MAGIC,
        )
        return "X3", s, mb

    def stage2b(st, g):
        """Second half of the finish chain, one iteration later, so each
        engine's per-iteration load stays uniform."""
        tag, t, mb = st
        if tag == "X1b":
            w = s_pool.tile([128, STRIP], F32, tag="s", name="w1")
            nc.gpsimd.tensor_scalar(
                w[:], t[:], MAGIC, MAGIC,
                op0=mybir.AluOpType.add, op1=mybir.AluOpType.subtract,
            )
            return "X1", w, mb
        if tag == "X45b":
            r = s_pool.tile([128, STRIP], F32, tag="s", name="r")
            nc.scalar.activation(
                r[:], t[:], mybir.ActivationFunctionType.Relu, bias=bnm1[:],
            )
            if g in X5:
                return "X5", r, mb
            w = s_pool.tile([128, STRIP], F32, tag="s", name="w4")
            nc.scalar.activation(
                w[:], r[:], mybir.ActivationFunctionType.Copy, bias=-1.0,
            )
            return "X4", w, mb
        return st

    def stage3(st, halves, k, g):
        """Final quant ops on Pool (adjacent in its queue), PE back-transposes
        into PSUM, ACT copies PSUM -> the output half tiles (strip 3 straddles
        both images, so its copy is split)."""
        mode, t, mb = st
        o = o_pool.tile([128, STRIP], F32, tag="o", name="o")
        if mode in ("X1", "X4"):
            nc.gpsimd.tensor_tensor(
                r4(o), r4(t), bc(mb, F32), op=mybir.AluOpType.mult,
            )
        elif mode == "X5":
            nc.vector.scalar_tensor_tensor(
                r4(o), r4(t), 1.0, bc(mb, F32),
                op0=mybir.AluOpType.subtract, op1=mybir.AluOpType.mult,
            )
        else:
            c = v_pool.tile([128, STRIP], F32, tag="v", name="c")
            nc.gpsimd.tensor_scalar(
                c[:], t[:], MAGIC - 1.0, MAGIC + 0.75,
                op0=mybir.AluOpType.max, op1=mybir.AluOpType.min,
            )
            nc.vector.scalar_tensor_tensor(
                r4(o), r4(c), MAGIC, bc(mb, F32),
                op0=mybir.AluOpType.subtract, op1=mybir.AluOpType.mult,
            )
        oT = pso_pool.tile([128, STRIP], F32, tag="oT", name="oT")
        for j in range(NBLK):
            nc.tensor.transpose(
                oT[:, j * 128 : j * 128 + 128],
                o[:, j * 128 : j * 128 + 128], ident[:],
            )
        # copy into per-image half tiles; cols [896k, 896k+896) of the pair
        c0, c1 = k * STRIP, (k + 1) * STRIP
        for h in range(2):
            lo, hi = max(c0, h * SP), min(c1, (h + 1) * SP)
            if lo < hi:
                nc.scalar.copy(
                    halves[h][:, lo - h * SP : hi - h * SP],
                    oT[:, lo - c0 : hi - c0],
                )

    def load_slab(sl, pieces):
        """Allocate slab sl's input tile and DMA it in `pieces` chunks per
        image (finer chunks let the first strips start sooner)."""
        pair, chh = divmod(sl, C // 128)
        x_sb = x_pool.tile([128, PAIR_F], F32, tag="x_sb", name="x_sb")
        q = SP // pieces
        for h in range(2):
            for p in range(pieces):
                nc.sync.dma_start(
                    out=x_sb[:, h * SP + p * q : h * SP + (p + 1) * q],
                    in_=x[2 * pair + h, 128 * chh : 128 * chh + 128,
                          p * q : (p + 1) * q],
                )
        return x_sb

    def store_half(half, sl, h, pieces=1):
        """Output DMA for image h of slab sl. Dispatched from the SP queue:
        all input DMAs are prefetched in the first iterations, so SP's FIFO
        is otherwise idle, and the dispatch's wait on the half's last copy
        would head-of-line block ACT's (depth-0) engine queue if issued
        there."""
        pair, chh = divmod(sl, C // 128)
        q = SP // pieces
        for p in range(pieces):
            nc.sync.dma_start(
                out=y[2 * pair + h, 128 * chh : 128 * chh + 128,
                      p * q : (p + 1) * q],
                in_=half[:, p * q : (p + 1) * q],
            )

    # One flattened 3-stage software pipeline over all 28 strips with a skew
    # of 2 iterations between stages — stage1(g) | stage2(g-2) | stage3(g-4)
    # — crossing slab boundaries, so ~5 strips are in flight and the long
    # cross-engine dependency chain of each strip is covered by the pipeline.
    x_slabs = {sl: load_slab(sl, 2 if sl == 0 else 1) for sl in range(2)}
    halves = {}
    st1 = {}
    st2 = {}
    def emit1(g):
        if not (0 <= g < TOT):
            return
        sl, k = divmod(g, NSTRIP)
        if k == 0:
            halves[sl] = [
                out_pool.tile([128, SP], F32, tag="oh", name="oh")
                for _ in range(2)
            ]
            if sl + 2 < NSLAB:
                x_slabs[sl + 2] = load_slab(sl + 2, 1)
        st1[g] = stage1(x_slabs[sl], k)
        if k == NSTRIP - 1:
            del x_slabs[sl]

    def emit2(g):
        if not (0 <= g < TOT):
            return
        st2[g] = stage2(st1.pop(g), g)

    def emit2b(g):
        if not (0 <= g < TOT):
            return
        st2[g] = stage2b(st2[g], g)

    def emit3(g):
        if not (0 <= g < TOT):
            return
        sl2, k2 = divmod(g, NSTRIP)
        stage3(st2.pop(g), halves[sl2], k2, g)
        if k2 == 3:
            store_half(halves[sl2][0], sl2, 0)
        elif k2 == NSTRIP - 1:
            store_half(
                halves[sl2][1], sl2, 1,
                pieces=4 if sl2 == NSLAB - 1 else 1,
            )
            del halves[sl2]

    for g in range(TOT + S3_GAP):
        for ch in ORDER:
            if ch == "1":
                emit1(g)
            elif ch == "2":
                emit2(g - S2_GAP)
            elif ch == "b":
                emit2b(g - S2B_GAP)
            else:
                emit3(g - S3_GAP)

    for p in (
        pso_pool, psx_pool, small_pool, o_pool, w_pool, s_pool, v_pool,
        u_pool, out_pool, x_pool, const_pool,
    ):
        p.release()


_CACHED = None


def _build():
    global _CACHED
    if _CACHED is None:
        nc = bacc.Bacc("TRN2", target_bir_lowering=False, debug=False)
        x = nc.dram_tensor("x", [NPC, C, SP], F32, kind="ExternalInput")
        y = nc.dram_tensor("y", [NPC, C, SP], F32, kind="ExternalOutput")
        with tile.TileContext(nc) as tc:
            bfp_body(tc, x[:], y[:])
        nc.compile()
        _CACHED = nc
    return _CACHED


def kernel(activations, mantissa_bits, blk, _trace=False, _tmpdir=None):
    mb = int(np.asarray(mantissa_bits))
    b = int(np.asarray(blk))
    assert mb == 3 and b == 32, (mb, b)
    x = np.ascontiguousarray(np.asarray(activations, dtype=np.float32))
    assert x.shape == (N, C, H, W), x.shape

    xs = x.reshape(N_CORES, NPC, C, SP)
    in_maps = [{"x": xs[k]} for k in range(N_CORES)]
    nc = _build()
    res = bass_utils.run_bass_kernel_spmd(
        nc, in_maps, core_ids=list(range(N_CORES)), trace=_trace, tmpdir=_tmpdir
    )
    outs = [np.asarray(res.results[k]["y"]) for k in range(N_CORES)]
    out = np.stack(outs, axis=0).reshape(N, C, H, W)
    if _trace:
        return out, res
    return out
